# revision 1
# baseline (speedup 1.0000x reference)
"""Causal multi-head attention (B=4, H=16, S=2048, D=64) on 8 TRN2 NeuronCores.

Sharding: the 64 (batch, head) pairs are fully independent; each core gets 8
pairs. q/k are pre-transposed to d-major [64, 2048] and converted to bf16 on
the host during sharding, so every DMA is contiguous and the TensorEngine
runs single-pass bf16 matmuls (fp32 matmuls cost two PE passes).

Per-core algorithm (flash-attention, transposed-score layout): TWO pairs are
processed in lockstep ("streams" X/Y) so the in-order PE queue always holds
matmuls that are independent of the other stream's pending exp — without
this the PE idles in ~400ns slivers every block waiting on ScalarE, which
also keeps the PE's HAM activity monitor from ever releasing the 1.2GHz
cold-clock throttle (measured: every matmul ran at the cold-clock rate).

Per q-tile ("unit", 512 q columns), per k-tile group of 2 (only k-tiles in
the causal lower triangle; diagonal blocks at reduced width), alternating
X/Y streams:
  S^T[k,q] = matmul(lhsT=K^T tile [64,128], rhs=Q^T tile [64,w])  (PSUM)
  P = exp(S^T * 1/8) -> bf16 SBUF             (ScalarE, PSUM->SBUF)
  diagonal blocks: zero the masked (q<k) triangle (GPSIMD affine_select)
  acc[65,512] += matmul(lhsT=V'[128,65], rhs=P)   V' has a ones column,
    so acc row 64 accumulates the softmax denominator for free.
PV groups trail their S^T group by one lockstep round.

Unit tails run when BOTH streams' accumulation groups are closed (emitting
other matmuls inside an open PSUM accumulation group deadlocks the HW):
  evict acc -> SBUF bf16 [65,512]; per 128-col q-block:
  tp[128,65] = matmul(lhsT=osb[65,128], rhs=I65)   (transpose via matmul:
    tp cols 0..63 = out[q,d] un-normalized, col 64 = denominator)
  rcol = 1/tp[:,64]; out = tp[:,0:64] * rcol       (per-partition scalar)
  DMA out[q-block, 64] fp32 to DRAM (contiguous)

Output per core is [8*2048, 64] in natural [q, d] layout; the host only
scatters slices back into the full [4, 2048, 1024] array.
"""

import math

import numpy as np
import ml_dtypes

import concourse.bass as bass
import concourse.bacc as bacc
import concourse.tile as tile
import concourse.mybir as mybir
from concourse import bass_utils
from concourse.masks import make_identity

B, H, S, D = 4, 16, 2048, 64
N_CORES = 8
PAIRS = (B * H) // N_CORES  # 8 pairs per core
QT = 512                    # q-tile width
KT = 128                    # k-tile rows (PE contraction tile)
NQT = S // QT               # 4 q-tiles per pair
GR = 2                      # k-tiles per pipeline group
SCALE = 1.0 / math.sqrt(D)
BF16 = ml_dtypes.bfloat16

_COMPILED = {}


def build_nc():
    nc = bacc.Bacc(
        "TRN2",
        target_bir_lowering=False,
        debug=False,
        enable_asserts=True,
        num_devices=N_CORES,
    )
    f32 = mybir.dt.float32
    bf16 = mybir.dt.bfloat16

    qt_d = nc.dram_tensor("qt", [PAIRS * D, S], bf16, kind="ExternalInput").ap()
    kt_d = nc.dram_tensor("kt", [PAIRS * D, S], bf16, kind="ExternalInput").ap()
    v_d = nc.dram_tensor("v", [PAIRS * S, D], bf16, kind="ExternalInput").ap()
    out_d = nc.dram_tensor("out", [PAIRS * S, D], f32, kind="ExternalOutput").ap()

    with tile.TileContext(nc) as tc:
        with (
            tc.tile_pool(name="consts", bufs=1) as consts,
            tc.tile_pool(name="qk", bufs=3) as qk_pool,
            tc.tile_pool(name="vp", bufs=3) as v_pool,
            tc.tile_pool(name="pp", bufs=12) as p_pool,
            tc.tile_pool(name="op", bufs=2) as o_pool,
            tc.tile_pool(name="fp", bufs=4) as f_pool,
            tc.tile_pool(name="rp", bufs=4) as r_pool,
            tc.tile_pool(name="ps", bufs=4, space="PSUM") as ps_pool,
            tc.tile_pool(name="acc", bufs=2, space="PSUM") as acc_pool,
            tc.tile_pool(name="tp", bufs=2, space="PSUM") as tp_pool,
        ):
            # I65: 65x65 identity for the transpose-matmul.
            ident = consts.tile([D + 1, D + 1], bf16)
            make_identity(nc, ident)

            def load_pair(p):
                qt_sb = qk_pool.tile([D, S], bf16, tag="qt", name=f"qt{p}")
                kt_sb = qk_pool.tile([D, S], bf16, tag="kt", name=f"kt{p}")
                nc.sync.dma_start(out=qt_sb, in_=qt_d[p * D:(p + 1) * D, :])
                nc.sync.dma_start(out=kt_sb, in_=kt_d[p * D:(p + 1) * D, :])
                v_sb = v_pool.tile([KT, S // KT, D + 1], bf16, tag="v",
                                   name=f"v{p}")
                nc.gpsimd.memset(v_sb[:, :, D:D + 1], 1.0)
                nc.sync.dma_start(
                    out=v_sb[:, :, 0:D],
                    in_=v_d[p * S:(p + 1) * S, :].rearrange(
                        "(t kp) d -> kp t d", kp=KT),
                )
                return qt_sb, kt_sb, v_sb

            def emit_st_group(sb, j, g):
                qt_sb, kt_sb, _ = sb
                tiles = []
                for half in range(GR):
                    t = GR * g + half
                    off = max(0, KT * t - QT * j)
                    w = QT - off
                    ps = ps_pool.tile([KT, QT], f32, tag="ps", name="ps")
                    nc.tensor.matmul(
                        ps[:, 0:w],
                        lhsT=kt_sb[:, KT * t:KT * (t + 1)],
                        rhs=qt_sb[:, QT * j + off:QT * (j + 1)],
                        start=True, stop=True,
                    )
                    p_sb = p_pool.tile([KT, QT], bf16, tag="p", name="p_sb")
                    nc.scalar.activation(
                        out=p_sb[:, 0:w], in_=ps[:, 0:w],
                        func=mybir.ActivationFunctionType.Exp,
                        scale=SCALE,
                    )
                    if t >= (QT // KT) * j:  # diagonal block: zero q < k
                        nc.gpsimd.affine_select(
                            out=p_sb[:, 0:w], in_=p_sb[:, 0:w],
                            compare_op=mybir.AluOpType.is_ge,
                            fill=0.0, base=0,
                            pattern=[[1, w]], channel_multiplier=-1,
                        )
                    tiles.append((p_sb, off))
                return tiles

            def emit_pv_group(sb, acc, nkt, g, tiles):
                v_sb = sb[2]
                for half in range(GR):
                    t = GR * g + half
                    p_sb, off = tiles[half]
                    nc.tensor.matmul(
                        acc[:, off:QT],
                        lhsT=v_sb[:, t, :],
                        rhs=p_sb[:, 0:QT - off],
                        start=(t == 0), stop=(t == nkt - 1),
                    )

            def emit_tail(p, j, acc):
                osb = o_pool.tile([D + 1, QT], bf16, tag="osb", name="osb")
                nc.vector.tensor_copy(osb, acc)
                for b in range(QT // KT):
                    tp = tp_pool.tile([KT, D + 1], f32, tag="tp", name="tp")
                    nc.tensor.matmul(
                        tp,
                        lhsT=osb[:, KT * b:KT * (b + 1)],
                        rhs=ident,
                        start=True, stop=True,
                    )
                    rcol = r_pool.tile([KT, 1], f32, tag="rc", name="rcol")
                    nc.vector.reciprocal(rcol, tp[:, D:D + 1])
                    fsb = f_pool.tile([KT, D], f32, tag="f", name="fsb")
                    nc.vector.tensor_scalar_mul(fsb, tp[:, 0:D], rcol)
                    row0 = p * S + QT * j + KT * b
                    nc.sync.dma_start(out=out_d[row0:row0 + KT, :], in_=fsb)

            pending_tails = []
            for pp in range(PAIRS // 2):  # lockstep pair-pair (X, Y)
                px, py = 2 * pp, 2 * pp + 1
                sbx = load_pair(px)
                sby = load_pair(py)
                for j in range(NQT):
                    nkt = (QT // KT) * (j + 1)
                    ngr = nkt // GR
                    accx = acc_pool.tile([D + 1, QT], f32, tag="acc",
                                         name="accx")
                    accy = acc_pool.tile([D + 1, QT], f32, tag="acc",
                                         name="accy")
                    pend = []  # [(stream, g, tiles)]
                    for g in range(ngr):
                        pend.append(("x", g, emit_st_group(sbx, j, g)))
                        if g == 0 and pending_tails:
                            for args in pending_tails:
                                emit_tail(*args)
                            pending_tails = []
                        pend.append(("y", g, emit_st_group(sby, j, g)))
                        while len(pend) > 2:
                            s, gg, tiles = pend.pop(0)
                            emit_pv_group(sbx if s == "x" else sby,
                                          accx if s == "x" else accy,
                                          nkt, gg, tiles)
                    for s, gg, tiles in pend:
                        emit_pv_group(sbx if s == "x" else sby,
                                      accx if s == "x" else accy,
                                      nkt, gg, tiles)
                    pending_tails = [(px, j, accx), (py, j, accy)]
            for args in pending_tails:
                emit_tail(*args)

    nc.compile()
    return nc


def _get_nc():
    if "nc" not in _COMPILED:
        _COMPILED["nc"] = build_nc()
    return _COMPILED["nc"]


def make_in_maps(q, k, v):
    q = np.asarray(q, dtype=np.float32).reshape(B * H, S, D)
    k = np.asarray(k, dtype=np.float32).reshape(B * H, S, D)
    v = np.asarray(v, dtype=np.float32).reshape(B * H, S, D)
    in_maps = []
    for c in range(N_CORES):
        sl = slice(c * PAIRS, (c + 1) * PAIRS)
        in_maps.append({
            "qt": np.ascontiguousarray(
                q[sl].transpose(0, 2, 1)).reshape(PAIRS * D, S).astype(BF16),
            "kt": np.ascontiguousarray(
                k[sl].transpose(0, 2, 1)).reshape(PAIRS * D, S).astype(BF16),
            "v": np.ascontiguousarray(v[sl]).reshape(PAIRS * S, D).astype(BF16),
        })
    return in_maps


def assemble(results):
    out = np.empty((B * H, S, D), dtype=np.float32)
    for c in range(N_CORES):
        out[c * PAIRS:(c + 1) * PAIRS] = results[c]["out"].reshape(PAIRS, S, D)
    return np.ascontiguousarray(
        out.reshape(B, H, S, D).transpose(0, 2, 1, 3).reshape(B, S, H * D))


def kernel(q, k, v):
    nc = _get_nc()
    res = bass_utils.run_bass_kernel_spmd(
        nc, make_in_maps(q, k, v), core_ids=list(range(N_CORES)))
    return assemble(res.results)



# revision 2
# speedup vs baseline: 1.9113x; 1.9113x over previous
"""Causal multi-head attention (B=4, H=16, S=2048, D=64) on 8 TRN2 NeuronCores.

Sharding: 64 (batch, head) pairs, 8 per core, processed as 4 "duos" (X, Y).
q/k are host-pre-transposed to d-major and duo-stacked: X's 64 d-rows on
SBUF partitions 0-63, Y's on 64-127.

Per-duo algorithm (flash-attention, transposed-score layout):

S^T stage — ROW-TILED matmul pairs: X's S^T at tile_position (0,0) using PE
rows 0-63, Y's at (64,0) using rows 64-127. The two contraction-64 matmuls
run concurrently in the PE array (~233ns/pair warm vs 2x535ns cold serial in
the v1 kernel): full-array activity also keeps the HAM clock monitor at
K=8/8 (2.4 GHz) — half-array streams measurably never warm up.

exp — one wide ACTIVATE per k-tile over the [128, 2, 512] two-bank PSUM
super-tile holding both streams' scores ((N+352)/1.2 ns: batching both
streams amortizes the 352-cycle ScalarE instruction overhead). ScalarE alone
(~46us/duo) would cap the kernel, so a fraction of k-tiles (EXP_PATTERN) is
computed on the otherwise-idle VectorE as a Schraudolph bit-trick exp:
round(x*128*log2e*scale + (16256 - 128*0.043677)) written as int16 and
bitcast to bf16 ≈ exp(x*scale) within ±3% (measured end-to-end rel-err
0.008 at a 0.02 gate). Causal masks: single GPSIMD affine_select over both
streams' first 128 columns of each diagonal tile.

PV stage — acc[65, 512] += V'[128, 65]^T @ P[128, w] per stream; V' carries
a ones column so acc row 64 accumulates the softmax denominator. PV lags
S^T by one GR=2 group so the in-order PE queue always holds independent work.

Unit tails (deferred into the next unit's first S-group for overlap):
evict acc -> SBUF bf16 (alternating ScalarE/VectorE), transpose via 4
identity matmuls into one PSUM bank [128, 4, 65], one strided reciprocal of
the 4 denominator columns, 4 per-partition-scalar normalize muls, one DMA of
[512, 64] fp32 to DRAM in natural [q, d] layout.
"""

import math

import numpy as np
import ml_dtypes

import concourse.bass as bass
import concourse.bacc as bacc
import concourse.tile as tile
import concourse.mybir as mybir
from concourse import bass_utils
from concourse.masks import make_identity

B, H, S, D = 4, 16, 2048, 64
N_CORES = 8
PAIRS = (B * H) // N_CORES  # 8 heads per core
DUOS = PAIRS // 2           # 4 lockstep duos per core
QT = 512                    # q-tile width
KT = 128                    # k-tile rows
NQT = S // QT               # 4 q-tiles per head
GR = 2                      # k-tiles per pipeline group
SCALE = 1.0 / math.sqrt(D)
A_SCH = (128.0 / math.log(2.0)) * SCALE       # Schraudolph slope (scale folded)
B_SCH = 16256.0 - 128.0 * 0.043677            # Schraudolph offset (bf16 bias)
EXP_PATTERN = ("s", "v", "s", "v", "s")       # 3/5 ScalarE exact, 2/5 DVE approx
BF16 = ml_dtypes.bfloat16

_COMPILED = {}


def build_nc():
    nc = bacc.Bacc(
        "TRN2",
        target_bir_lowering=False,
        debug=False,
        enable_asserts=True,
        num_devices=N_CORES,
    )
    f32 = mybir.dt.float32
    bf16 = mybir.dt.bfloat16
    i16 = mybir.dt.int16

    qt_d = nc.dram_tensor("qt", [DUOS * 2 * D, S], bf16, kind="ExternalInput").ap()
    kt_d = nc.dram_tensor("kt", [DUOS * 2 * D, S], bf16, kind="ExternalInput").ap()
    v_d = nc.dram_tensor("v", [PAIRS * S, D], bf16, kind="ExternalInput").ap()
    out_d = nc.dram_tensor("out", [PAIRS * S, D], f32, kind="ExternalOutput").ap()

    with tile.TileContext(nc) as tc:
        with (
            tc.tile_pool(name="consts", bufs=1) as consts,
            tc.tile_pool(name="qk", bufs=2) as qk_pool,
            tc.tile_pool(name="vp", bufs=2) as v_pool,
            tc.tile_pool(name="pp", bufs=6) as p_pool,
            tc.tile_pool(name="op", bufs=3) as o_pool,
            tc.tile_pool(name="fp", bufs=3) as f_pool,
            tc.tile_pool(name="rp", bufs=3) as r_pool,
            tc.tile_pool(name="big", bufs=2, space="PSUM") as big_pool,
            tc.tile_pool(name="acc", bufs=2, space="PSUM") as acc_pool,
            tc.tile_pool(name="tpp", bufs=2, space="PSUM") as tp_pool,
        ):
            ident = consts.tile([D + 1, D + 1], bf16)
            make_identity(nc, ident)
            st = {"exp": 0, "tail": 0}

            def load_duo(dd):
                qsb = qk_pool.tile([2 * D, S], bf16, tag="qsb", name=f"q{dd}")
                ksb = qk_pool.tile([2 * D, S], bf16, tag="ksb", name=f"k{dd}")
                nc.sync.dma_start(out=qsb, in_=qt_d[dd * 128:(dd + 1) * 128, :])
                nc.sync.dma_start(out=ksb, in_=kt_d[dd * 128:(dd + 1) * 128, :])
                vs = []
                for s_ in range(2):
                    h = 2 * dd + s_
                    vt = v_pool.tile([KT, S // KT, D + 1], bf16, tag=f"v{s_}",
                                     name=f"v{dd}_{s_}")
                    nc.gpsimd.memset(vt[:, :, D:D + 1], 1.0)
                    nc.sync.dma_start(
                        out=vt[:, :, 0:D],
                        in_=v_d[h * S:(h + 1) * S, :].rearrange(
                            "(t kp) d -> kp t d", kp=KT),
                    )
                    vs.append(vt)
                return qsb, ksb, vs

            def emit_s(sb, j, t):
                qsb, ksb, _ = sb
                off = max(0, KT * t - QT * j)
                w = QT - off
                q0 = QT * j + off
                ps = big_pool.tile([KT, 2, QT], f32, tag="ps", name="ps")
                nc.tensor.matmul(
                    ps[:, 0, 0:w],
                    lhsT=ksb[0:D, KT * t:KT * (t + 1)],
                    rhs=qsb[0:D, q0:QT * (j + 1)],
                    start=True, stop=True, tile_position=(0, 0),
                )
                nc.tensor.matmul(
                    ps[:, 1, 0:w],
                    lhsT=ksb[D:2 * D, KT * t:KT * (t + 1)],
                    rhs=qsb[D:2 * D, q0:QT * (j + 1)],
                    start=True, stop=True, tile_position=(64, 0),
                )
                p3 = p_pool.tile([KT, 2, QT], bf16, tag="p3", name="p3")
                eng = EXP_PATTERN[st["exp"] % len(EXP_PATTERN)]
                st["exp"] += 1
                if eng == "s":
                    nc.scalar.activation(
                        out=p3[:, :, 0:w], in_=ps[:, :, 0:w],
                        func=mybir.ActivationFunctionType.Exp, scale=SCALE,
                    )
                else:
                    nc.vector.tensor_scalar(
                        out=p3[:, :, 0:w].bitcast(i16), in0=ps[:, :, 0:w],
                        scalar1=A_SCH, scalar2=B_SCH,
                        op0=mybir.AluOpType.mult, op1=mybir.AluOpType.add,
                    )
                if t >= (QT // KT) * j:  # diagonal tile: zero q_rel < k_rel
                    nc.gpsimd.affine_select(
                        out=p3[:, :, 0:KT], in_=p3[:, :, 0:KT],
                        compare_op=mybir.AluOpType.is_ge,
                        fill=0.0, base=0,
                        pattern=[[0, 2], [1, KT]], channel_multiplier=-1,
                    )
                return p3, off, w

            def emit_pv(sb, accs, nkt, t, p3off):
                p3, off, w = p3off
                for s_ in range(2):
                    nc.tensor.matmul(
                        accs[s_][:, off:QT],
                        lhsT=sb[2][s_][:, t, :],
                        rhs=p3[:, s_, 0:w],
                        start=(t == 0), stop=(t == nkt - 1),
                    )

            def emit_tail(h, j, acc):
                osb = o_pool.tile([D + 1, QT], bf16, tag="osb", name="osb")
                if st["tail"] % 2 == 0:
                    nc.vector.tensor_copy(osb, acc)
                else:
                    nc.scalar.copy(out=osb, in_=acc)
                st["tail"] += 1
                tp = tp_pool.tile([KT, QT // KT, D + 1], f32, tag="tp",
                                  name="tp")
                for b_ in range(QT // KT):
                    nc.tensor.matmul(
                        tp[:, b_, :],
                        lhsT=osb[:, KT * b_:KT * (b_ + 1)],
                        rhs=ident, start=True, stop=True,
                    )
                rinv = r_pool.tile([KT, QT // KT], f32, tag="ri", name="rinv")
                nc.vector.reciprocal(rinv, tp[:, :, D])
                fsb = f_pool.tile([KT, QT // KT, D], f32, tag="f", name="fsb")
                for b_ in range(QT // KT):
                    nc.vector.tensor_scalar_mul(
                        fsb[:, b_, :], tp[:, b_, 0:D], rinv[:, b_:b_ + 1])
                row0 = h * S + QT * j
                nc.sync.dma_start(
                    out=out_d[row0:row0 + QT, :].rearrange(
                        "(b p) d -> p b d", p=KT),
                    in_=fsb,
                )

            sbs = load_duo(0)
            sbs_next = None
            pending_tails = []
            for dd in range(DUOS):
                sb = sbs
                for j in range(NQT):
                    nkt = (QT // KT) * (j + 1)
                    ngr = nkt // GR
                    accx = acc_pool.tile([D + 1, QT], f32, tag="acc",
                                         name="accx")
                    accy = acc_pool.tile([D + 1, QT], f32, tag="acc",
                                         name="accy")
                    accs = (accx, accy)
                    pend = []
                    for g in range(ngr):
                        for half in range(GR):
                            t = GR * g + half
                            pend.append((t, emit_s(sb, j, t)))
                        if g == 0:
                            if pending_tails:
                                for args in pending_tails:
                                    emit_tail(*args)
                                pending_tails = []
                            if j == 1 and dd + 1 < DUOS:
                                sbs_next = load_duo(dd + 1)
                        while len(pend) > GR:
                            t, p3off = pend.pop(0)
                            emit_pv(sb, accs, nkt, t, p3off)
                    for t, p3off in pend:
                        emit_pv(sb, accs, nkt, t, p3off)
                    pending_tails = [(2 * dd, j, accx), (2 * dd + 1, j, accy)]
                sbs = sbs_next
            for args in pending_tails:
                emit_tail(*args)

    nc.compile()
    return nc


def _get_nc():
    if "nc" not in _COMPILED:
        _COMPILED["nc"] = build_nc()
    return _COMPILED["nc"]


def make_in_maps(q, k, v):
    q = np.asarray(q, dtype=np.float32).reshape(B * H, S, D)
    k = np.asarray(k, dtype=np.float32).reshape(B * H, S, D)
    v = np.asarray(v, dtype=np.float32).reshape(B * H, S, D)
    in_maps = []
    for c in range(N_CORES):
        sl = slice(c * PAIRS, (c + 1) * PAIRS)
        # duo-stacked d-major [DUOS*128, S]: duo dd rows 0-63 = head 2dd,
        # rows 64-127 = head 2dd+1
        qt = np.ascontiguousarray(q[sl].transpose(0, 2, 1)).reshape(
            DUOS * 2 * D, S)
        kt = np.ascontiguousarray(k[sl].transpose(0, 2, 1)).reshape(
            DUOS * 2 * D, S)
        in_maps.append({
            "qt": qt.astype(BF16),
            "kt": kt.astype(BF16),
            "v": np.ascontiguousarray(v[sl]).reshape(PAIRS * S, D).astype(BF16),
        })
    return in_maps


def assemble(results):
    out = np.empty((B * H, S, D), dtype=np.float32)
    for c in range(N_CORES):
        out[c * PAIRS:(c + 1) * PAIRS] = results[c]["out"].reshape(PAIRS, S, D)
    return np.ascontiguousarray(
        out.reshape(B, H, S, D).transpose(0, 2, 1, 3).reshape(B, S, H * D))


def kernel(q, k, v):
    nc = _get_nc()
    res = bass_utils.run_bass_kernel_spmd(
        nc, make_in_maps(q, k, v), core_ids=list(range(N_CORES)))
    return assemble(res.results)
